# revision 56
# baseline (speedup 1.0000x reference)
"""Farthest-point-sampling (npoint=2) Bass kernel for Trainium2 — v6.

Problem: xyz [1, 64, 3, 262144] fp32 -> indices [64, 2] (int64 on host).
Per batch b:
  idx0 = argmax_n y[n]            (y = coord plane 1)
  c    = (x,y,z)[idx0]
  idx1 = argmax_n ((x-cx)^2 + (y-cy)^2 + (z-cz)^2)
argmax = first occurrence on ties (jnp.argmax semantics).

Sharding: data-parallel over batch; 8 NeuronCores x 8 batches each.

v6 design (exact fp32 end-to-end):
- Streaming: per-batch segmented VectorE max-reduce ([128,2048] ->
  [128,128] segment maxes of 16) + max8/find_index8 on the segmax row
  giving per-partition (max, first-seg).
- Distance adds split 3-way: PE identity-matmul cols 0:512 (PSUM),
  VectorE cols 512:1152, GpSimd cols 1152:2048 (all exact fp32).
- ONE 8-batch finale per phase, in a partition-per-batch [8,16] layout:
  exact cross-partition first-occurrence cell select (PE transpose of
  (max, cellcode) pairs), one contiguous-run indirect gather ([8,16]
  y-runs / [24,16] xyz-runs), transpose-free scans; the centroid is
  selected from the gathered xyz with a one-hot matmul; the distance
  rescue recomputes the 16 candidates in the reference ((x+y)+z) order.
  The y finale fully precedes the distance phase; the distance finale's
  gather queues behind all GpSimd adds (no head-of-line blocking).
"""

import numpy as np

import concourse.bacc as bacc
import concourse.bass as bass
import concourse.mybir as mybir
from concourse.masks import make_identity
from concourse.tile import TileContext

B = 64  # full batch
N_CORES = 8
BPC = B // N_CORES  # batches per core (also the finale group size)
N = 262144
P = 128
COLS = N // P       # 2048
SEG = 16            # points per segment
NSEG = COLS // SEG  # 128 segments per partition
PECOLS = 512        # PE add range
VCOLS = 640         # VectorE add range (512:1152)
GCOLS = COLS - PECOLS - VCOLS  # GpSimd range (1152:2048)
Q = 3 * BPC         # 24 gather rows (q = 3*b + c)

F32 = mybir.dt.float32
U32 = mybir.dt.uint32
I32 = mybir.dt.int32
AX = mybir.AxisListType.X
OP = mybir.AluOpType
SQUARE = mybir.ActivationFunctionType.Square


def build_nc():
    nc = bacc.Bacc()
    xin = nc.dram_tensor("xyz", [BPC, 3, N], F32, kind="ExternalInput")
    out = nc.dram_tensor("idx", [1, 2 * BPC], I32, kind="ExternalOutput")
    xflat = xin.rearrange("b c n -> (b c n)")[:, None]

    with TileContext(nc) as tc:
        with (
            tc.tile_pool(name="consts", bufs=1) as consts,
            tc.tile_pool(name="acc", bufs=1) as acc,
            tc.tile_pool(name="ypool", bufs=BPC) as ypool,
            tc.tile_pool(name="xzpool", bufs=3) as xzpool,
            tc.tile_pool(name="sqpool", bufs=2) as sqpool,
            tc.tile_pool(name="dpool", bufs=2) as dpool,
            tc.tile_pool(name="finpool", bufs=2) as finpool,
            tc.tile_pool(name="psda", bufs=2, space="PSUM") as psda_pool,
            tc.tile_pool(name="pssm", bufs=2, space="PSUM") as pssm,
        ):
            # ---------------- constants ----------------
            ident = consts.tile([P, P], F32)
            make_identity(nc, ident)

            def iota_f32(shape, tag, pattern, base, cm):
                ti = consts.tile(shape, I32, tag=tag + "_i")
                nc.gpsimd.iota(ti, pattern=pattern, base=base, channel_multiplier=cm)
                tf = consts.tile(shape, F32, tag=tag)
                nc.vector.tensor_copy(tf, ti)
                return tf

            wpb2 = iota_f32([P, 1], "wpb2", [[0, 1]], N, -COLS)     # N - 2048p
            wj8 = iota_f32([BPC, SEG], "wj8", [[-1, SEG]], SEG, 0)  # 16-j rows
            cn3val = iota_f32([3, 1], "cn3val", [[0, 1]], 0, N)     # c*N
            bidx8 = iota_f32([BPC, 1], "bidx8", [[0, 1]], 0, 1)     # 0..7
            pidx8 = iota_f32([BPC, 1], "pidx8", [[0, 1]], 0, 1)
            pidx3 = iota_f32([3, 1], "pidx3", [[0, 1]], 0, 1)
            fdiv3_24 = iota_f32([BPC, BPC, 3], "fdiv3_24", [[1, BPC], [0, 3]], 0, 0)
            fm3_24 = iota_f32([3, BPC, 3], "fm3_24", [[0, BPC], [1, 3]], 0, 0)
            f8_24 = iota_f32([Q, BPC], "f8_24", [[1, BPC]], 0, 0)
            # yoff8[b] = b*3N + N
            yoff8 = iota_f32([BPC, 1], "yoff8", [[0, 1]], N, 3 * N)

            w3blk = consts.tile([BPC, Q], F32)     # W[b, 3b+c] = 1
            nc.vector.tensor_tensor(
                out=w3blk, in0=fdiv3_24.rearrange("p a b -> p (a b)"),
                in1=pidx8.to_broadcast([BPC, Q]), op=OP.is_equal,
            )
            wc3 = consts.tile([3, Q], F32)         # W[c, 3b+c] = 1
            nc.vector.tensor_tensor(
                out=wc3, in0=fm3_24.rearrange("p a b -> p (a b)"),
                in1=pidx3.to_broadcast([3, Q]), op=OP.is_equal,
            )

            ones128 = consts.tile([1, P], F32)
            nc.vector.memset(ones128, 1.0)
            ones8x16 = consts.tile([BPC, SEG], F32)
            nc.vector.memset(ones8x16, 1.0)

            # --- init-time composed constants (via PE selector matmuls) ---
            spi = pssm.tile([P, 512], F32, tag="smallps")
            nc.tensor.matmul(spi[0:Q, 8:9], w3blk, bidx8, start=True, stop=True)
            bl24 = consts.tile([Q, 1], F32)        # q // 3
            nc.scalar.copy(bl24, spi[0:Q, 8:9])
            nc.tensor.matmul(spi[0:Q, 16:17], wc3, cn3val, start=True, stop=True)
            cN24 = consts.tile([Q, 1], F32)        # (q % 3) * N
            nc.scalar.copy(cN24, spi[0:Q, 16:17])
            # w24to8[q, b] = 1 if q//3 == b
            w24to8 = consts.tile([Q, BPC], F32)
            nc.vector.tensor_tensor(
                out=w24to8, in0=f8_24,
                in1=bl24.to_broadcast([Q, BPC]), op=OP.is_equal,
            )
            # c24[q] = (q%3)*N + (q//3)*3N
            c24 = consts.tile([Q, 1], F32)
            nc.vector.tensor_scalar_mul(c24, bl24, float(3 * N))
            nc.vector.tensor_add(c24, c24, cN24)

            # per-partition (top8 vals, top8 seg idx) per batch, y and dist
            yv8 = acc.tile([P, 8 * BPC], F32)
            yi8 = acc.tile([P, 8 * BPC], U32)
            dv8 = acc.tile([P, 8 * BPC], F32)
            di8 = acc.tile([P, 8 * BPC], U32)
            negc24_t = acc.tile([Q, 1], F32)
            negc128_all = acc.tile([P, BPC, 3], F32)
            out_i = acc.tile([1, 2 * BPC], I32)

            def col0(t8):
                return t8.rearrange("p (b k) -> p b k", k=8)[:, :, 0]

            # ---------------- DMA ----------------
            tys = [None] * BPC
            txzs = [None] * BPC

            def dma_y(b):
                ty = ypool.tile([P, COLS], F32, tag="ty")
                tys[b] = ty
                nc.sync.dma_start(ty, xin[b, 1].rearrange("(p m) -> p m", p=P))

            def dma_xz(b):
                txz = xzpool.tile([P, 2, COLS], F32, tag="txz")
                txzs[b] = txz
                nc.sync.dma_start(
                    txz, xin[b, 0::2].rearrange("c (p m) -> p c m", p=P)
                )

            # ---------------- building blocks ----------------
            def yseg_reduce(b):
                yseg = finpool.tile([P, NSEG], F32, tag="yseg")
                nc.vector.tensor_reduce(
                    yseg,
                    tys[b].rearrange("p (s j) -> p s j", j=SEG),
                    axis=AX, op=OP.max,
                )
                nc.vector.max(out=yv8[:, 8 * b : 8 * b + 8], in_=yseg)
                nc.vector.max_index(
                    yi8[:, 8 * b : 8 * b + 8], yv8[:, 8 * b : 8 * b + 8], yseg
                )

            def locate8(tag, v8, i8):
                """Exact first-occurrence argmax cell per batch (all 8).
                Returns base8 [BPC,1] = p*2048 + s*16 per batch, and sp."""
                cb8 = finpool.tile([P, BPC], F32, tag=f"cb{tag}")
                nc.vector.scalar_tensor_tensor(
                    out=cb8, in0=col0(i8), scalar=float(-SEG),
                    in1=wpb2.to_broadcast([P, BPC]), op0=OP.mult, op1=OP.add,
                )
                sp = pssm.tile([P, 512], F32, tag="smallps")
                nc.tensor.transpose(sp[0:BPC, 0:P], col0(v8), ident)
                nc.tensor.transpose(sp[0:BPC, 128:256], cb8, ident)
                rowv = finpool.tile([BPC, P], F32, tag=f"rowv{tag}")
                nc.vector.tensor_copy(rowv, sp[0:BPC, 0:P])
                rowc = finpool.tile([BPC, P], F32, tag=f"rowc{tag}")
                nc.vector.tensor_copy(rowc, sp[0:BPC, 128:256])
                m8 = finpool.tile([BPC, 1], F32, tag=f"m{tag}")
                nc.vector.tensor_reduce(m8, rowv, axis=AX, op=OP.max)
                candc = finpool.tile([BPC, P], F32, tag=f"candc{tag}")
                nc.vector.scalar_tensor_tensor(
                    out=candc, in0=rowv, scalar=m8, in1=rowc,
                    op0=OP.is_equal, op1=OP.mult,
                )
                bcode = finpool.tile([BPC, 1], F32, tag=f"bcode{tag}")
                nc.vector.tensor_reduce(bcode, candc, axis=AX, op=OP.max)
                base8 = finpool.tile([BPC, 1], F32, tag=f"base{tag}")
                nc.vector.tensor_scalar(
                    out=base8, in0=bcode, scalar1=-1.0, scalar2=float(N),
                    op0=OP.mult, op1=OP.add,
                )
                return base8, sp

            def gather24(tag, base8, sp):
                """xyz runs of the winning cells: [24,16], q = 3b+c."""
                nc.tensor.matmul(
                    sp[0:Q, 280:281], w3blk, base8, start=True, stop=True
                )
                b24 = finpool.tile([Q, 1], F32, tag=f"b24{tag}")
                nc.vector.tensor_copy(b24, sp[0:Q, 280:281])
                offs24 = finpool.tile([Q, 1], U32, tag=f"o24{tag}")
                nc.vector.tensor_add(offs24, c24, b24)
                xyz24 = finpool.tile([Q, SEG], F32, tag=f"x24{tag}")
                nc.gpsimd.indirect_dma_start(
                    out=xyz24, out_offset=None, in_=xflat,
                    in_offset=bass.IndirectOffsetOnAxis(ap=offs24, axis=0),
                )
                return xyz24

            def scan8(tag, rows8, base8):
                """Per-batch first-occurrence argmax within each [8,16] row.
                Returns (icol [8,1] global index, oh8 [8,16] one-hot)."""
                rmax = finpool.tile([BPC, 1], F32, tag=f"srm{tag}")
                nc.vector.tensor_reduce(rmax, rows8, axis=AX, op=OP.max)
                jcand = finpool.tile([BPC, SEG], F32, tag=f"sjc{tag}")
                nc.vector.scalar_tensor_tensor(
                    out=jcand, in0=rows8, scalar=rmax, in1=wj8,
                    op0=OP.is_equal, op1=OP.mult,
                )
                jcode = finpool.tile([BPC, 1], F32, tag=f"sjq{tag}")
                nc.vector.tensor_reduce(jcode, jcand, axis=AX, op=OP.max)
                oh8 = finpool.tile([BPC, SEG], F32, tag=f"soh{tag}")
                nc.vector.scalar_tensor_tensor(
                    out=oh8, in0=jcand, scalar=jcode, in1=ones8x16,
                    op0=OP.is_equal, op1=OP.mult,
                )
                jstar = finpool.tile([BPC, 1], F32, tag=f"sjs{tag}")
                nc.vector.tensor_scalar(
                    out=jstar, in0=jcode, scalar1=-1.0, scalar2=float(SEG),
                    op0=OP.mult, op1=OP.add,
                )
                icol = finpool.tile([BPC, 1], F32, tag=f"sic{tag}")
                nc.vector.tensor_add(icol, jstar, base8)
                return icol, oh8

            def write_out(tag, icol, sp, out_off):
                nc.tensor.transpose(sp[0:1, 488:496], icol, ident[0:BPC, 0:BPC])
                irow = finpool.tile([1, BPC], F32, tag=f"irow{tag}")
                nc.vector.tensor_copy(irow, sp[0:1, 488:496])
                nc.scalar.copy(out_i[0:1, out_off : out_off + BPC], irow)

            # ---------------- y finale (all 8 batches) ----------------
            def y_finale():
                base8, sp = locate8("y", yv8, yi8)
                offs8 = finpool.tile([BPC, 1], U32, tag="yo8")
                nc.vector.tensor_add(offs8, yoff8, base8)
                yrows = finpool.tile([BPC, SEG], F32, tag="yrows")
                nc.gpsimd.indirect_dma_start(
                    out=yrows, out_offset=None, in_=xflat,
                    in_offset=bass.IndirectOffsetOnAxis(ap=offs8, axis=0),
                )
                xyz24 = gather24("y", base8, sp)
                yidx, oh8 = scan8("y", yrows, base8)
                # centroid select from xyz24 via one-hot rows
                nc.tensor.matmul(sp[0:Q, 288:304], w3blk, oh8, start=True, stop=True)
                prod = finpool.tile([Q, SEG], F32, tag="cprod")
                nc.vector.tensor_mul(prod, xyz24, sp[0:Q, 288:304])
                cval = finpool.tile([Q, 1], F32, tag="cval")
                nc.vector.tensor_reduce(cval, prod, axis=AX, op=OP.add)
                nc.vector.tensor_scalar_mul(negc24_t, cval, -1.0)
                nc.tensor.transpose(sp[0:1, 304:328], negc24_t, ident[0:Q, 0:Q])
                negcrow = finpool.tile([1, Q], F32, tag="ncrow")
                nc.vector.tensor_copy(negcrow, sp[0:1, 304:328])
                nc.tensor.matmul(
                    sp[:, 328:352], ones128, negcrow, start=True, stop=True,
                )
                nc.scalar.copy(
                    negc128_all.rearrange("p b c -> p (b c)"), sp[:, 328:352]
                )
                write_out("y", yidx, sp, 0)

            # ---------------- distance per batch ----------------
            def dist_batch(b):
                ty, txz = tys[b], txzs[b]
                sqz = sqpool.tile([P, COLS], F32, tag="sqz")
                nc.scalar.activation(
                    sqz, txz[:, 1], SQUARE, bias=negc128_all[:, b, 2:3]
                )
                sqx = sqpool.tile([P, COLS], F32, tag="sqx")
                nc.scalar.activation(
                    sqx, txz[:, 0], SQUARE, bias=negc128_all[:, b, 0:1]
                )
                sqy = sqpool.tile([P, COLS], F32, tag="sqy")
                nc.scalar.activation(
                    sqy, ty, SQUARE, bias=negc128_all[:, b, 1:2]
                )
                # cols 0:512 on PE (exact fp32 accumulate in PSUM)
                psda = psda_pool.tile([P, PECOLS], F32, tag="psda")
                for i, sq in enumerate((sqz, sqx, sqy)):
                    nc.tensor.matmul(
                        psda, ident, sq[:, 0:PECOLS], start=(i == 0), stop=(i == 2)
                    )
                dseg = finpool.tile([P, NSEG], F32, tag="dseg")
                if b < BPC - 1:
                    nc.vector.tensor_reduce(
                        dseg[:, 0 : PECOLS // SEG],
                        psda.rearrange("p (s j) -> p s j", j=SEG),
                        axis=AX, op=OP.max,
                    )
                # cols 512:1152 on VectorE, 1152:2048 on GpSimd.
                # Last batch: the +sqy add (critical tail path) runs as one
                # wide VectorE op; GpSimd only does sqz+sqx (overlaps sqy).
                dv = dpool.tile([P, VCOLS + GCOLS], F32, tag="dv")
                VS = slice(PECOLS, PECOLS + VCOLS)
                GS = slice(PECOLS + VCOLS, COLS)
                nc.vector.tensor_add(dv[:, 0:VCOLS], sqz[:, VS], sqx[:, VS])
                nc.gpsimd.tensor_add(dv[:, VCOLS:], sqz[:, GS], sqx[:, GS])
                if b == BPC - 1:
                    nc.vector.tensor_add(dv, dv, sqy[:, PECOLS:])
                else:
                    nc.vector.tensor_add(
                        dv[:, 0:VCOLS], dv[:, 0:VCOLS], sqy[:, VS]
                    )
                    nc.gpsimd.tensor_add(
                        dv[:, VCOLS:], dv[:, VCOLS:], sqy[:, GS]
                    )
                if b == BPC - 1:
                    nc.vector.tensor_reduce(
                        dseg[:, 0 : PECOLS // SEG],
                        psda.rearrange("p (s j) -> p s j", j=SEG),
                        axis=AX, op=OP.max,
                    )
                nc.vector.tensor_reduce(
                    dseg[:, PECOLS // SEG : NSEG],
                    dv.rearrange("p (s j) -> p s j", j=SEG),
                    axis=AX, op=OP.max,
                )
                nc.vector.max(out=dv8[:, 8 * b : 8 * b + 8], in_=dseg)
                nc.vector.max_index(
                    di8[:, 8 * b : 8 * b + 8], dv8[:, 8 * b : 8 * b + 8], dseg
                )

            # ---------------- dist finale (all 8 batches) ----------------
            def d_finale():
                base8, sp = locate8("d", dv8, di8)
                xyz24 = gather24("d", base8, sp)
                sub = finpool.tile([Q, SEG], F32, tag="dsub")
                nc.vector.tensor_add(
                    sub, xyz24, negc24_t.to_broadcast([Q, SEG])
                )
                sq = finpool.tile([Q, SEG], F32, tag="dsq")
                nc.vector.tensor_mul(sq, sub, sub)
                nc.tensor.matmul(sp[0:BPC, 352:368], w24to8, sq, start=True, stop=True)
                d8 = finpool.tile([BPC, SEG], F32, tag="d8x16")
                nc.vector.tensor_copy(d8, sp[0:BPC, 352:368])
                didx, _ = scan8("d", d8, base8)
                write_out("d", didx, sp, BPC)

            # ---------------- emission ----------------
            for b in range(BPC):
                dma_y(b)
            for b in range(BPC):
                dma_xz(b)

            for b in range(BPC):
                yseg_reduce(b)
            y_finale()
            for b in range(BPC):
                dist_batch(b)
            d_finale()

            nc.sync.dma_start(out[:, :], out_i[:, :])

    nc.compile()
    return nc


_NC_CACHE = None


def _get_nc():
    global _NC_CACHE
    if _NC_CACHE is None:
        _NC_CACHE = build_nc()
    return _NC_CACHE


def kernel(xyz: np.ndarray) -> np.ndarray:
    from concourse.bass_utils import run_bass_kernel_spmd

    assert xyz.shape == (1, B, 3, N), xyz.shape
    xyz = np.ascontiguousarray(xyz, dtype=np.float32)
    nc = _get_nc()
    in_maps = [
        {"xyz": np.ascontiguousarray(xyz[0, k * BPC : (k + 1) * BPC])}
        for k in range(N_CORES)
    ]
    res = run_bass_kernel_spmd(nc, in_maps, core_ids=list(range(N_CORES)))
    # out layout per core: [1, 16] = [idx0 x8 | idx1 x8]
    outs = [res.results[k]["idx"].reshape(2, BPC).T for k in range(N_CORES)]
    return np.concatenate(outs, axis=0).astype(np.int64)
